# revision 4
# baseline (speedup 1.0000x reference)
"""GQA attention (B=2, S=2048, D=2048, Hq=16, Hkv=4, hd=128) on 8 TRN2 cores.

Sharding: core c = b*4 + kv handles batch b and kv-head kv (with its 4 query
heads). Each core computes its partial output (A_heads @ Wo_slice); the host
sums the 4 partials per batch and adds the bias.

Per-core kernel (PE-bound design, ~247us of fp32r matmul at full rate):
  pass A: single x pass -> K^T, V (PE-transposed), and Q^T for ALL 4 heads
          kept resident in SBUF; Q psum->sbuf copies ride the Activation
          engine, K/V copies on DVE, so pass A is PE/DMA-paced.
  pass B: per 512-query block ib: per head, 8 slots of [2x S^T matmul into a
          2-bank PSUM pair -> one wide exp (bf16 out, scale folded) -> 2x PV
          matmul (moving operand bf16, full rate)], with the output-projection
          matmuls of block ib-1 interleaved 2-per-slot so the PE never idles
          while ACT computes exps. Softmax denominators: bf16 add tree on DVE
          (2x mode) + gpsimd partition_all_reduce; normalize folded into the
          PSUM->SBUF copy of O^T. Out-proj PSUM tiles stream to DRAM via
          gpsimd copies + DMA.
"""
import sys

sys.path.insert(0, "/opt/trn_rl_repo")
import numpy as np

B, S, D = 2, 2048, 2048
HQ, HKV, HD = 16, 4, 128
G = HQ // HKV
SCALE = HD ** -0.5
P = 128
NB = 512
DC = D // P     # 16 contraction chunks
SB = S // NB    # 4 seq blocks of 512
ST = S // P     # 16 seq tiles of 128
JP = ST // 2    # 8 j-pairs per head
DP = 2          # PV pair lookahead (pipeline depth, in pairs)

_CACHE = {}


def _build(reps=(1, 1, 1)):
    from contextlib import ExitStack, nullcontext

    import concourse.bacc as bacc
    import concourse.bass_isa as bass_isa
    import concourse.mybir as mybir
    import concourse.tile as tile
    from concourse.masks import make_identity

    F32 = mybir.dt.float32
    F32R = mybir.dt.float32r
    BF16 = mybir.dt.bfloat16
    Exp = mybir.ActivationFunctionType.Exp

    nc = bacc.Bacc("TRN2", target_bir_lowering=False, debug=False)
    # inputs arrive host-pre-tiled to SBUF layout (partition dim first) so
    # each tensor loads in O(1) DMA instructions (HWDGE issue is 625ns each)
    xT = nc.dram_tensor("xT", [P, DC, S], F32R, kind="ExternalInput").ap()
    wq = nc.dram_tensor("wq", [P, DC, G * HD], F32R, kind="ExternalInput").ap()
    wk = nc.dram_tensor("wk", [P, DC, HD], F32R, kind="ExternalInput").ap()
    wv = nc.dram_tensor("wv", [P, DC, HD], F32R, kind="ExternalInput").ap()
    wo = nc.dram_tensor("wo", [P, G, D], F32R, kind="ExternalInput").ap()
    out = nc.dram_tensor("out", [S, D], F32, kind="ExternalOutput").ap()

    r1, r2, _ = reps

    with tile.TileContext(nc) as tc, ExitStack() as stk:
        persist = stk.enter_context(tc.tile_pool(name="persist", bufs=1))
        kt_sb = persist.tile([P, S], F32R)
        v_sb = persist.tile([P, ST, HD], BF16)
        qt_sb = persist.tile([P, G, S], F32R)

        def _loop(r):
            return tc.For_i(0, r, 1) if r > 1 else nullcontext()

        # ---- pass A: K^T, V and Q^T for the whole sequence ----
        with ExitStack() as pas:
            a1 = pas.enter_context(tc.tile_pool(name="a1", bufs=1))
            xta_pool = pas.enter_context(tc.tile_pool(name="xta", bufs=2))
            vt_pool = pas.enter_context(tc.tile_pool(name="vt", bufs=2))
            ps_kv = pas.enter_context(tc.tile_pool(name="ps_kv", bufs=2, space="PSUM"))
            ps_t = pas.enter_context(tc.tile_pool(name="ps_t", bufs=2, space="PSUM"))
            ps_q = pas.enter_context(tc.tile_pool(name="ps_q", bufs=2, space="PSUM"))

            wk_sb = a1.tile([P, DC, HD], F32R)
            wv_sb = a1.tile([P, DC, HD], F32R)
            wq_sb = a1.tile([P, DC, G * HD], F32R)
            ident = a1.tile([P, P], F32)
            make_identity(nc, ident)
            # transfer order on the shared DMA path decides when block-0
            # compute can start: Wk first, x block 0 in pipelined quarter
            # loads, Wv before the V chain, Wq last (needed ~20us in)
            nc.sync.dma_start(out=wk_sb, in_=wk)

            with _loop(r1):
              for xb in range(SB):
                cols = slice(xb * NB, (xb + 1) * NB)
                xt = xta_pool.tile([P, DC, NB], F32R, name="xt")
                if xb == 0:
                    for g in range(4):
                        cg = slice(4 * g, 4 * g + 4)
                        nc.sync.dma_start(out=xt[:, cg, :], in_=xT[:, cg, cols])
                    for g in range(4):
                        cg = slice(4 * g, 4 * g + 4)
                        nc.sync.dma_start(out=wq_sb[:, cg, :], in_=wq[:, cg, :])
                    nc.sync.dma_start(out=wv_sb, in_=wv)
                else:
                    nc.sync.dma_start(out=xt, in_=xT[:, :, cols])

                def k_chain():
                    pk = ps_kv.tile([P, NB], F32, name="pk")
                    for c in range(DC):
                        nc.tensor.matmul(pk, wk_sb[:, c, :], xt[:, c, :],
                                         start=(c == 0), stop=(c == DC - 1))
                    nc.vector.tensor_copy(out=kt_sb[:, cols], in_=pk)

                def v_chain():
                    pv = ps_kv.tile([P, NB], F32, name="pv")
                    for c in range(DC):
                        nc.tensor.matmul(pv, wv_sb[:, c, :], xt[:, c, :],
                                         start=(c == 0), stop=(c == DC - 1))
                    vt = vt_pool.tile([P, NB], F32, name="vt")
                    nc.vector.tensor_copy(out=vt, in_=pv)
                    for k in range(NB // P):
                        pt = ps_t.tile([P, P], F32, name="pt")
                        nc.tensor.transpose(pt, vt[:, k * P:(k + 1) * P], ident)
                        nc.vector.tensor_copy(out=v_sb[:, xb * (NB // P) + k, :],
                                              in_=pt)

                def q_chains():
                    for h in range(G):
                        pq = ps_q.tile([P, NB], F32, name="pq")
                        for c in range(DC):
                            nc.tensor.matmul(pq, wq_sb[:, c, h * HD:(h + 1) * HD],
                                             xt[:, c, :], start=(c == 0),
                                             stop=(c == DC - 1))
                        # ACT copies Q so DVE keeps headroom for pass B's trees
                        nc.scalar.copy(out=qt_sb[:, h, cols], in_=pq)

                # block 0 follows DMA arrival order (wk, x, wq, wv)
                k_chain()
                if xb == 0:
                    q_chains()
                    v_chain()
                else:
                    v_chain()
                    q_chains()

        # ---- pass B: attention with interleaved output projection ----
        with ExitStack() as pbs:
            b1 = pbs.enter_context(tc.tile_pool(name="b1", bufs=1))
            wo_sb = b1.tile([P, G, D], F32R)
            nc.sync.dma_start(out=wo_sb, in_=wo)
            ot_pool = pbs.enter_context(tc.tile_pool(name="ot", bufs=2))
            ex_pool = pbs.enter_context(tc.tile_pool(name="ex", bufs=2))
            dn_pool = pbs.enter_context(tc.tile_pool(name="dn", bufs=2))
            st_pool = pbs.enter_context(tc.tile_pool(name="st", bufs=2))
            ps_s = pbs.enter_context(tc.tile_pool(name="ps_s", bufs=2, space="PSUM"))
            ps_o = pbs.enter_context(tc.tile_pool(name="ps_o", bufs=2, space="PSUM"))
            ps_p = pbs.enter_context(tc.tile_pool(name="ps_p", bufs=2, space="PSUM"))

            def oproj_ops(ib):
                """Generator yielding the 64 out-proj matmuls (+ tail copy and
                per-row DMA after each 4-mm chain) for query block ib."""
                for t in range(4):
                    it = 4 * ib + t
                    ot_ib = ot_tiles[ib % 2]
                    so = st_pool.tile([P, D], F32, name="so")
                    for nb in range(D // NB):
                        pso = ps_p.tile([P, NB], F32, name="pso")
                        for h in range(G):
                            yield ("mm", pso, ot_ib, so, h, t, nb, it)

            def oproj_step(state, tail=False):
                it = state.get("iter")
                if it is None:
                    return
                try:
                    op, pso, ot_ib, so, h, t, nb, it_row = next(it)
                except StopIteration:
                    state["iter"] = None
                    return
                nc.tensor.matmul(pso, ot_ib[:, h, t * P:(t + 1) * P],
                                 wo_sb[:, h, nb * NB:(nb + 1) * NB],
                                 start=(h == 0), stop=(h == G - 1))
                if h == G - 1:
                    n = state["chain"] = state.get("chain", 0) + 1
                    # PSUM->SBUF copy engine (gpsimd can't read PSUM): DVE
                    # while ACT is busy with exps; alternate DVE/ACT in the
                    # tail so bank release never paces the PE
                    dst = so[:, nb * NB:(nb + 1) * NB]
                    if tail and n % 2 == 0:
                        nc.scalar.copy(out=dst, in_=pso)
                    else:
                        nc.vector.tensor_copy(out=dst, in_=pso)
                    if tail and it_row >= S // P - 2:
                        # last rows: per-chunk writes so the final drain is
                        # one 512-col DMA, not a full row
                        nc.sync.dma_start(
                            out=out[it_row * P:(it_row + 1) * P,
                                    nb * NB:(nb + 1) * NB], in_=dst)
                    elif nb == D // NB - 1:
                        nc.sync.dma_start(
                            out=out[it_row * P:(it_row + 1) * P, :], in_=so)

            with _loop(r2):
              ot_tiles = [None, None]
              for ib in range(SB):
                icols = slice(ib * NB, (ib + 1) * NB)
                ot_tiles[ib % 2] = ot_pool.tile([P, G, NB], F32R, name="ot")
                ot_ib = ot_tiles[ib % 2]
                # out-proj of the previous block, spread ~1.6 matmuls/slot so
                # the feed lasts the whole block (64 mms over 40 slots)
                ostate = {"iter": iter(oproj_ops(ib - 1)) if ib > 0 else None}
                nslots = G * (JP + DP)
                kslot = 0
                for h in range(G):
                    ex = ex_pool.tile([P, ST, NB], BF16, name="ex")
                    po = ps_o.tile([P, NB], F32, name="po")
                    gc = dn_pool.tile([P, 4, NB], BF16, name="gc")
                    for jj in range(JP + DP):
                        if jj < JP:
                            pss = ps_s.tile([P, 2, NB], F32, name="pss")
                            for u in range(2):
                                j = 2 * jj + u
                                nc.tensor.matmul(pss[:, u, :],
                                                 kt_sb[:, j * P:(j + 1) * P],
                                                 qt_sb[:, h, icols],
                                                 start=True, stop=True)
                            nc.scalar.activation(out=ex[:, 2 * jj:2 * jj + 2, :],
                                                 in_=pss, func=Exp, scale=SCALE)
                        if jj < JP and jj % 2 == 1:
                            # collapse exp pair group g as soon as it's ready:
                            # keeps the post-last-exp denominator path short
                            g = jj // 2
                            tg = dn_pool.tile([P, 2, NB], BF16, name="tg")
                            nc.vector.tensor_add(out=tg, in0=ex[:, 4 * g:4 * g + 2, :],
                                                 in1=ex[:, 4 * g + 2:4 * g + 4, :])
                            nc.vector.tensor_add(out=gc[:, g, :], in0=tg[:, 0, :],
                                                 in1=tg[:, 1, :])
                            if g == 1:
                                nc.vector.tensor_add(out=gc[:, 0, :],
                                                     in0=gc[:, 0, :], in1=gc[:, 1, :])
                        kslot += 1
                        nfeed = (kslot * 64) // nslots - ((kslot - 1) * 64) // nslots
                        for _ in range(nfeed):
                            oproj_step(ostate)
                        jc = jj - DP
                        if jc >= 0:
                            for u in range(2):
                                j = 2 * jc + u
                                nc.tensor.matmul(po, v_sb[:, j, :], ex[:, j, :],
                                                 start=(j == 0), stop=(j == ST - 1))
                    # denominator tail: fold group 3, reduce, reciprocal
                    nc.vector.tensor_add(out=gc[:, 2, :], in0=gc[:, 2, :],
                                         in1=gc[:, 3, :])
                    dsum = dn_pool.tile([P, NB], F32, name="dsum")
                    nc.vector.tensor_add(out=dsum, in0=gc[:, 0, :], in1=gc[:, 2, :])
                    dred = dn_pool.tile([P, NB], F32, name="dred")
                    nc.gpsimd.partition_all_reduce(dred, dsum, P, bass_isa.ReduceOp.add)
                    nc.vector.reciprocal(out=dsum, in_=dred)
                    nc.vector.tensor_tensor(out=ot_ib[:, h, :], in0=po,
                                            in1=dsum, op=mybir.AluOpType.mult)
                # drain any out-proj steps not absorbed by the slots
                while ostate["iter"] is not None:
                    oproj_step(ostate)
              # tail: out-proj of the last block
              ostate = {"iter": iter(oproj_ops(SB - 1))}
              while ostate["iter"] is not None:
                  oproj_step(ostate, tail=True)

    nc.compile()
    return nc


def _get_nc():
    if "nc" not in _CACHE:
        _CACHE["nc"] = _build()
    return _CACHE["nc"]


def timed_runner(reps):
    nc = _build(reps)
    return make_runner(nc)


def make_runner(nc, n_cores=8):
    """Persistent jitted SPMD runner (mirrors bass2jax.run_bass_via_pjrt's
    multi-core path, without donation so the executable can be re-invoked on
    device-resident inputs for timing)."""
    import jax
    from jax.experimental.shard_map import shard_map
    from jax.sharding import Mesh, PartitionSpec

    import concourse.mybir as mybir
    from concourse import bass2jax

    bass2jax.install_neuronx_cc_hook()
    partition_name = nc.partition_id_tensor.name if nc.partition_id_tensor else None
    in_names, out_names, out_avals, zero_shapes = [], [], [], []
    for alloc in nc.m.functions[0].allocations:
        if not isinstance(alloc, mybir.MemoryLocationSet):
            continue
        name = alloc.memorylocations[0].name
        if alloc.kind == "ExternalInput":
            if name != partition_name:
                in_names.append(name)
        elif alloc.kind == "ExternalOutput":
            out_names.append(name)
            shape = tuple(alloc.tensor_shape)
            dtype = mybir.dt.np(alloc.dtype)
            out_avals.append(jax.core.ShapedArray(shape, dtype))
            zero_shapes.append((shape, dtype))
    n_params = len(in_names)
    all_in_names = tuple(in_names + out_names)
    if partition_name is not None:
        all_in_names = all_in_names + (partition_name,)

    def _body(*args):
        operands = list(args)
        if partition_name is not None:
            operands.append(bass2jax.partition_id_tensor())
        outs = bass2jax._bass_exec_p.bind(
            *operands,
            out_avals=tuple(out_avals),
            in_names=all_in_names,
            out_names=tuple(out_names),
            lowering_input_output_aliases=(),
            sim_require_finite=True,
            sim_require_nnan=True,
            nc=nc,
        )
        return tuple(outs)

    devices = jax.devices()[:n_cores]
    mesh = Mesh(np.asarray(devices), ("core",))
    n_outs = len(out_names)
    fn = jax.jit(
        shard_map(_body, mesh=mesh,
                  in_specs=(PartitionSpec("core"),) * (n_params + n_outs),
                  out_specs=(PartitionSpec("core"),) * n_outs,
                  check_rep=False),
        keep_unused=True,
    )
    return fn, in_names, out_names, zero_shapes, mesh


def _get_runner():
    if "runner" not in _CACHE:
        _CACHE["runner"] = make_runner(_get_nc())
    return _CACHE["runner"]


def run_cores(in_maps):
    """Run the 8-core SPMD program; returns list of per-core {name: array}."""
    import jax

    fn, in_names, out_names, zero_shapes, mesh = _get_runner()
    n = len(in_maps)
    concat_in = [np.concatenate([np.asarray(in_maps[c][nm]) for c in range(n)], axis=0)
                 for nm in in_names]
    concat_zero = [np.zeros((n * s[0], *s[1:]), dt) for s, dt in zero_shapes]
    outs = fn(*concat_in, *concat_zero)
    outs = [np.asarray(o) for o in outs]
    return [
        {nm: outs[i].reshape(n, *zero_shapes[i][0])[c] for i, nm in enumerate(out_names)}
        for c in range(n)
    ]


def _ptile(a, dtype=None):
    """[R, C] -> [128, R//128, C]: partition-major retiling for O(1)-DMA loads."""
    R, C = a.shape
    t = a.reshape(R // P, P, C).transpose(1, 0, 2)
    if dtype is not None:
        t = t.astype(dtype)
    return np.ascontiguousarray(t)


def shard_inputs(x, Wq, Wk, Wv, Wo):
    in_maps = []
    for b in range(B):
        xTb = _ptile(np.ascontiguousarray(x[b].T))
        for kv in range(HKV):
            in_maps.append({
                "xT": xTb,
                "wq": _ptile(Wq[:, kv * G * HD:(kv + 1) * G * HD]),
                "wk": _ptile(Wk[:, kv * HD:(kv + 1) * HD]),
                "wv": _ptile(Wv[:, kv * HD:(kv + 1) * HD]),
                "wo": _ptile(Wo[kv * G * HD:(kv + 1) * G * HD, :]),
            })
    return in_maps


def kernel(x, Wq, Wk, Wv, Wo, bo):
    x = np.asarray(x, np.float32)
    Wq = np.asarray(Wq, np.float32)
    Wk = np.asarray(Wk, np.float32)
    Wv = np.asarray(Wv, np.float32)
    Wo = np.asarray(Wo, np.float32)
    bo = np.asarray(bo, np.float32)
    results = run_cores(shard_inputs(x, Wq, Wk, Wv, Wo))
    out = np.empty((B, S, D), np.float32)
    for b in range(B):
        out[b] = results[4 * b]["out"]
        for kv in range(1, HKV):
            out[b] += results[4 * b + kv]["out"]
        out[b] += bo
    return out


# revision 5
# speedup vs baseline: 1.0441x; 1.0441x over previous
"""GQA attention (B=2, S=2048, D=2048, Hq=16, Hkv=4, hd=128) on 8 TRN2 cores.

Sharding: core c = b*4 + kv handles batch b and kv-head kv (with its 4 query
heads). Each core computes its partial output (A_heads @ Wo_slice); the host
sums the 4 partials per batch and adds the bias.

Per-core kernel (PE-bound design, ~247us of fp32r matmul at full rate):
  pass A: single x pass -> K^T, V (PE-transposed), and Q^T for ALL 4 heads
          kept resident in SBUF; Q psum->sbuf copies ride the Activation
          engine, K/V copies on DVE, so pass A is PE/DMA-paced.
  pass B: per 512-query block ib: per head, 8 slots of [2x S^T matmul into a
          2-bank PSUM pair -> one wide exp (bf16 out, scale folded) -> 2x PV
          matmul (moving operand bf16, full rate)], with the output-projection
          matmuls of block ib-1 interleaved 2-per-slot so the PE never idles
          while ACT computes exps. Softmax denominators: bf16 add tree on DVE
          (2x mode) + gpsimd partition_all_reduce; normalize folded into the
          PSUM->SBUF copy of O^T. Out-proj PSUM tiles stream to DRAM via
          gpsimd copies + DMA.
"""
import sys

sys.path.insert(0, "/opt/trn_rl_repo")
import numpy as np

B, S, D = 2, 2048, 2048
HQ, HKV, HD = 16, 4, 128
G = HQ // HKV
SCALE = HD ** -0.5
P = 128
NB = 512
DC = D // P     # 16 contraction chunks
SB = S // NB    # 4 seq blocks of 512
ST = S // P     # 16 seq tiles of 128
JP = ST // 2    # 8 j-pairs per head
DP = 3          # PV pair lookahead (pipeline depth, in pairs)
WIDE_EXP = True # exp over a [128,1024] 2-bank PSUM pair vs two [128,512]
ABLATE = 0      # timing ablation: 0 full, 1 no-oproj, 2 no-pv/den, 3 st+exp only, 4 st only

_CACHE = {}


def _build(reps=(1, 1, 1)):
    from contextlib import ExitStack, nullcontext

    import concourse.bacc as bacc
    import concourse.bass_isa as bass_isa
    import concourse.mybir as mybir
    import concourse.tile as tile
    from concourse.masks import make_identity

    F32 = mybir.dt.float32
    F32R = mybir.dt.float32r
    BF16 = mybir.dt.bfloat16
    Exp = mybir.ActivationFunctionType.Exp

    nc = bacc.Bacc("TRN2", target_bir_lowering=False, debug=False)
    # inputs arrive host-pre-tiled to SBUF layout (partition dim first) so
    # each tensor loads in O(1) DMA instructions (HWDGE issue is 625ns each)
    xT = nc.dram_tensor("xT", [P, DC, S], F32R, kind="ExternalInput").ap()
    wq = nc.dram_tensor("wq", [P, DC, G * HD], F32R, kind="ExternalInput").ap()
    wk = nc.dram_tensor("wk", [P, DC, HD], F32R, kind="ExternalInput").ap()
    wv = nc.dram_tensor("wv", [P, DC, HD], F32R, kind="ExternalInput").ap()
    wo = nc.dram_tensor("wo", [P, G, D], F32R, kind="ExternalInput").ap()
    out = nc.dram_tensor("out", [S, D], F32, kind="ExternalOutput").ap()

    r1, r2, _ = reps

    with tile.TileContext(nc) as tc, ExitStack() as stk:
        persist = stk.enter_context(tc.tile_pool(name="persist", bufs=1))
        kt_sb = persist.tile([P, S], F32R)
        v_sb = persist.tile([P, ST, HD], BF16)
        qt_sb = persist.tile([P, G, S], F32R)

        def _loop(r):
            return tc.For_i(0, r, 1) if r > 1 else nullcontext()

        # ---- pass A: K^T, V and Q^T for the whole sequence ----
        with ExitStack() as pas:
            a1 = pas.enter_context(tc.tile_pool(name="a1", bufs=1))
            xta_pool = pas.enter_context(tc.tile_pool(name="xta", bufs=2))
            vt_pool = pas.enter_context(tc.tile_pool(name="vt", bufs=2))
            ps_kv = pas.enter_context(tc.tile_pool(name="ps_kv", bufs=2, space="PSUM"))
            ps_t = pas.enter_context(tc.tile_pool(name="ps_t", bufs=2, space="PSUM"))
            ps_q = pas.enter_context(tc.tile_pool(name="ps_q", bufs=2, space="PSUM"))

            wk_sb = a1.tile([P, DC, HD], F32R)
            wv_sb = a1.tile([P, DC, HD], F32R)
            wq_sb = a1.tile([P, DC, G * HD], F32R)
            ident = a1.tile([P, P], F32)
            make_identity(nc, ident)
            # transfer order on the shared DMA path decides when block-0
            # compute can start: Wk first, x block 0 in pipelined quarter
            # loads, Wv before the V chain, Wq last (needed ~20us in)
            with _loop(r1):
              for xb in range(SB):
                cols = slice(xb * NB, (xb + 1) * NB)
                xt = xta_pool.tile([P, DC, NB], F32R, name="xt")
                if xb == 0:
                    nc.sync.dma_start(out=wk_sb, in_=wk)
                    for g in range(4):
                        cg = slice(4 * g, 4 * g + 4)
                        nc.sync.dma_start(out=xt[:, cg, :], in_=xT[:, cg, cols])
                    for g in range(8):
                        cg = slice(2 * g, 2 * g + 2)
                        nc.sync.dma_start(out=wq_sb[:, cg, :], in_=wq[:, cg, :])
                    nc.sync.dma_start(out=wv_sb, in_=wv)
                else:
                    nc.sync.dma_start(out=xt, in_=xT[:, :, cols])

                def k_chain():
                    pk = ps_kv.tile([P, NB], F32, name="pk")
                    for c in range(DC):
                        nc.tensor.matmul(pk, wk_sb[:, c, :], xt[:, c, :],
                                         start=(c == 0), stop=(c == DC - 1))
                    nc.vector.tensor_copy(out=kt_sb[:, cols], in_=pk)

                def v_chain():
                    pv = ps_kv.tile([P, NB], F32, name="pv")
                    for c in range(DC):
                        nc.tensor.matmul(pv, wv_sb[:, c, :], xt[:, c, :],
                                         start=(c == 0), stop=(c == DC - 1))
                    vt = vt_pool.tile([P, NB], F32, name="vt")
                    nc.vector.tensor_copy(out=vt, in_=pv)
                    for k in range(NB // P):
                        pt = ps_t.tile([P, P], F32, name="pt")
                        nc.tensor.transpose(pt, vt[:, k * P:(k + 1) * P], ident)
                        nc.vector.tensor_copy(out=v_sb[:, xb * (NB // P) + k, :],
                                              in_=pt)

                def q_chains():
                    for h in range(G):
                        pq = ps_q.tile([P, NB], F32, name="pq")
                        for c in range(DC):
                            nc.tensor.matmul(pq, wq_sb[:, c, h * HD:(h + 1) * HD],
                                             xt[:, c, :], start=(c == 0),
                                             stop=(c == DC - 1))
                        # ACT copies Q so DVE keeps headroom for pass B's trees
                        nc.scalar.copy(out=qt_sb[:, h, cols], in_=pq)

                # block 0 follows DMA arrival order (wk, x, wq, wv)
                k_chain()
                if xb == 0:
                    q_chains()
                    v_chain()
                else:
                    v_chain()
                    q_chains()

        # ---- pass B: attention with interleaved output projection ----
        with ExitStack() as pbs:
            b1 = pbs.enter_context(tc.tile_pool(name="b1", bufs=1))
            wo_sb = b1.tile([P, G, D], F32R)
            nc.sync.dma_start(out=wo_sb, in_=wo)
            ot_pool = pbs.enter_context(tc.tile_pool(name="ot", bufs=2))
            ex_pool = pbs.enter_context(tc.tile_pool(name="ex", bufs=2))
            dn_pool = pbs.enter_context(tc.tile_pool(name="dn", bufs=2))
            st_pool = pbs.enter_context(tc.tile_pool(name="st", bufs=2))
            ps_s = pbs.enter_context(tc.tile_pool(name="ps_s", bufs=2, space="PSUM"))
            ps_o = pbs.enter_context(tc.tile_pool(name="ps_o", bufs=2, space="PSUM"))
            ps_p = pbs.enter_context(tc.tile_pool(name="ps_p", bufs=2, space="PSUM"))

            def oproj_ops(ib):
                """Generator yielding the 64 out-proj matmuls (+ tail copy and
                per-row DMA after each 4-mm chain) for query block ib."""
                for t in range(4):
                    it = 4 * ib + t
                    ot_ib = ot_tiles[ib % 2]
                    so = st_pool.tile([P, D], F32, name="so")
                    for nb in range(D // NB):
                        pso = ps_p.tile([P, NB], F32, name="pso")
                        for h in range(G):
                            yield ("mm", pso, ot_ib, so, h, t, nb, it)

            def oproj_step(state, tail=False):
                it = state.get("iter")
                if it is None:
                    return
                try:
                    op, pso, ot_ib, so, h, t, nb, it_row = next(it)
                except StopIteration:
                    state["iter"] = None
                    return
                nc.tensor.matmul(pso, ot_ib[:, h, t * P:(t + 1) * P],
                                 wo_sb[:, h, nb * NB:(nb + 1) * NB],
                                 start=(h == 0), stop=(h == G - 1))
                if h == G - 1:
                    n = state["chain"] = state.get("chain", 0) + 1
                    # PSUM->SBUF copy engine (gpsimd can't read PSUM): DVE
                    # while ACT is busy with exps; alternate DVE/ACT in the
                    # tail so bank release never paces the PE
                    dst = so[:, nb * NB:(nb + 1) * NB]
                    if tail and n % 2 == 0:
                        nc.scalar.copy(out=dst, in_=pso)
                    else:
                        nc.vector.tensor_copy(out=dst, in_=pso)
                    if tail and it_row >= S // P - 2:
                        # last rows: per-chunk writes so the final drain is
                        # one 512-col DMA, not a full row
                        nc.sync.dma_start(
                            out=out[it_row * P:(it_row + 1) * P,
                                    nb * NB:(nb + 1) * NB], in_=dst)
                    elif nb == D // NB - 1:
                        nc.sync.dma_start(
                            out=out[it_row * P:(it_row + 1) * P, :], in_=so)

            with _loop(r2):
              ot_tiles = [None, None]
              for ib in range(SB):
                icols = slice(ib * NB, (ib + 1) * NB)
                ot_tiles[ib % 2] = (ot_pool.tile([P, G, NB], F32R, name="ot")
                                    if ABLATE < 2 else None)
                ot_ib = ot_tiles[ib % 2]
                # out-proj of the previous block, spread ~1.6 matmuls/slot so
                # the feed lasts the whole block (64 mms over 40 slots)
                ostate = {"iter": iter(oproj_ops(ib - 1)) if ib > 0 else None}
                nslots = G * (JP + DP)
                kslot = 0
                for h in range(G):
                    ex = (ex_pool.tile([P, ST, NB], BF16, name="ex")
                          if ABLATE < 4 else None)
                    po = ps_o.tile([P, NB], F32, name="po") if ABLATE < 2 else None
                    gc = (dn_pool.tile([P, 4, NB], BF16, name="gc")
                          if ABLATE < 2 else None)
                    for jj in range(JP + DP):
                        if jj < JP:
                            pss = ps_s.tile([P, 2, NB], F32, name="pss")
                            for u in range(2):
                                j = 2 * jj + u
                                nc.tensor.matmul(pss[:, u, :],
                                                 kt_sb[:, j * P:(j + 1) * P],
                                                 qt_sb[:, h, icols],
                                                 start=True, stop=True)
                            if ABLATE >= 4:
                                pass
                            elif WIDE_EXP:
                                nc.scalar.activation(out=ex[:, 2 * jj:2 * jj + 2, :],
                                                     in_=pss, func=Exp, scale=SCALE)
                            else:
                                for u in range(2):
                                    nc.scalar.activation(
                                        out=ex[:, 2 * jj + u, :],
                                        in_=pss[:, u, :], func=Exp, scale=SCALE)
                        if jj < JP and jj % 2 == 1 and ABLATE < 2:
                            # collapse exp pair group g as soon as it's ready:
                            # keeps the post-last-exp denominator path short
                            g = jj // 2
                            tg = dn_pool.tile([P, 2, NB], BF16, name="tg")
                            nc.vector.tensor_add(out=tg, in0=ex[:, 4 * g:4 * g + 2, :],
                                                 in1=ex[:, 4 * g + 2:4 * g + 4, :])
                            nc.vector.tensor_add(out=gc[:, g, :], in0=tg[:, 0, :],
                                                 in1=tg[:, 1, :])
                            if g == 1:
                                nc.vector.tensor_add(out=gc[:, 0, :],
                                                     in0=gc[:, 0, :], in1=gc[:, 1, :])
                        kslot += 1
                        nfeed = (kslot * 64) // nslots - ((kslot - 1) * 64) // nslots
                        if ABLATE == 0:
                            for _ in range(nfeed):
                                oproj_step(ostate)
                        jc = jj - DP
                        if jc >= 0 and ABLATE < 2:
                            for u in range(2):
                                j = 2 * jc + u
                                nc.tensor.matmul(po, v_sb[:, j, :], ex[:, j, :],
                                                 start=(j == 0), stop=(j == ST - 1))
                    if ABLATE >= 2:
                        continue
                    # denominator tail: fold group 3, reduce, reciprocal
                    nc.vector.tensor_add(out=gc[:, 2, :], in0=gc[:, 2, :],
                                         in1=gc[:, 3, :])
                    dsum = dn_pool.tile([P, NB], F32, name="dsum")
                    nc.vector.tensor_add(out=dsum, in0=gc[:, 0, :], in1=gc[:, 2, :])
                    dred = dn_pool.tile([P, NB], F32, name="dred")
                    nc.gpsimd.partition_all_reduce(dred, dsum, P, bass_isa.ReduceOp.add)
                    nc.vector.reciprocal(out=dsum, in_=dred)
                    nc.vector.tensor_tensor(out=ot_ib[:, h, :], in0=po,
                                            in1=dsum, op=mybir.AluOpType.mult)
                # drain any out-proj steps not absorbed by the slots
                while ABLATE == 0 and ostate["iter"] is not None:
                    oproj_step(ostate)
              # tail: out-proj of the last block
              if ABLATE == 0:
                  ostate = {"iter": iter(oproj_ops(SB - 1))}
                  while ostate["iter"] is not None:
                      oproj_step(ostate, tail=True)

    nc.compile()
    return nc


def _get_nc():
    if "nc" not in _CACHE:
        _CACHE["nc"] = _build()
    return _CACHE["nc"]


def timed_runner(reps):
    nc = _build(reps)
    return make_runner(nc)


def make_runner(nc, n_cores=8):
    """Persistent jitted SPMD runner (mirrors bass2jax.run_bass_via_pjrt's
    multi-core path, without donation so the executable can be re-invoked on
    device-resident inputs for timing)."""
    import jax
    from jax.experimental.shard_map import shard_map
    from jax.sharding import Mesh, PartitionSpec

    import concourse.mybir as mybir
    from concourse import bass2jax

    bass2jax.install_neuronx_cc_hook()
    partition_name = nc.partition_id_tensor.name if nc.partition_id_tensor else None
    in_names, out_names, out_avals, zero_shapes = [], [], [], []
    for alloc in nc.m.functions[0].allocations:
        if not isinstance(alloc, mybir.MemoryLocationSet):
            continue
        name = alloc.memorylocations[0].name
        if alloc.kind == "ExternalInput":
            if name != partition_name:
                in_names.append(name)
        elif alloc.kind == "ExternalOutput":
            out_names.append(name)
            shape = tuple(alloc.tensor_shape)
            dtype = mybir.dt.np(alloc.dtype)
            out_avals.append(jax.core.ShapedArray(shape, dtype))
            zero_shapes.append((shape, dtype))
    n_params = len(in_names)
    all_in_names = tuple(in_names + out_names)
    if partition_name is not None:
        all_in_names = all_in_names + (partition_name,)

    def _body(*args):
        operands = list(args)
        if partition_name is not None:
            operands.append(bass2jax.partition_id_tensor())
        outs = bass2jax._bass_exec_p.bind(
            *operands,
            out_avals=tuple(out_avals),
            in_names=all_in_names,
            out_names=tuple(out_names),
            lowering_input_output_aliases=(),
            sim_require_finite=True,
            sim_require_nnan=True,
            nc=nc,
        )
        return tuple(outs)

    devices = jax.devices()[:n_cores]
    mesh = Mesh(np.asarray(devices), ("core",))
    n_outs = len(out_names)
    fn = jax.jit(
        shard_map(_body, mesh=mesh,
                  in_specs=(PartitionSpec("core"),) * (n_params + n_outs),
                  out_specs=(PartitionSpec("core"),) * n_outs,
                  check_rep=False),
        keep_unused=True,
    )
    return fn, in_names, out_names, zero_shapes, mesh


def _get_runner():
    if "runner" not in _CACHE:
        _CACHE["runner"] = make_runner(_get_nc())
    return _CACHE["runner"]


def run_cores(in_maps):
    """Run the 8-core SPMD program; returns list of per-core {name: array}."""
    import jax

    fn, in_names, out_names, zero_shapes, mesh = _get_runner()
    n = len(in_maps)
    concat_in = [np.concatenate([np.asarray(in_maps[c][nm]) for c in range(n)], axis=0)
                 for nm in in_names]
    concat_zero = [np.zeros((n * s[0], *s[1:]), dt) for s, dt in zero_shapes]
    outs = fn(*concat_in, *concat_zero)
    outs = [np.asarray(o) for o in outs]
    return [
        {nm: outs[i].reshape(n, *zero_shapes[i][0])[c] for i, nm in enumerate(out_names)}
        for c in range(n)
    ]


def _ptile(a, dtype=None):
    """[R, C] -> [128, R//128, C]: partition-major retiling for O(1)-DMA loads."""
    R, C = a.shape
    t = a.reshape(R // P, P, C).transpose(1, 0, 2)
    if dtype is not None:
        t = t.astype(dtype)
    return np.ascontiguousarray(t)


def shard_inputs(x, Wq, Wk, Wv, Wo):
    in_maps = []
    for b in range(B):
        xTb = _ptile(np.ascontiguousarray(x[b].T))
        for kv in range(HKV):
            in_maps.append({
                "xT": xTb,
                "wq": _ptile(Wq[:, kv * G * HD:(kv + 1) * G * HD]),
                "wk": _ptile(Wk[:, kv * HD:(kv + 1) * HD]),
                "wv": _ptile(Wv[:, kv * HD:(kv + 1) * HD]),
                "wo": _ptile(Wo[kv * G * HD:(kv + 1) * G * HD, :]),
            })
    return in_maps


def kernel(x, Wq, Wk, Wv, Wo, bo):
    x = np.asarray(x, np.float32)
    Wq = np.asarray(Wq, np.float32)
    Wk = np.asarray(Wk, np.float32)
    Wv = np.asarray(Wv, np.float32)
    Wo = np.asarray(Wo, np.float32)
    bo = np.asarray(bo, np.float32)
    results = run_cores(shard_inputs(x, Wq, Wk, Wv, Wo))
    out = np.empty((B, S, D), np.float32)
    for b in range(B):
        out[b] = results[4 * b]["out"]
        for kv in range(1, HKV):
            out[b] += results[4 * b + kv]["out"]
        out[b] += bo
    return out
